# revision 105
# baseline (speedup 1.0000x reference)
# Bidirectional VSSM (4-direction selective scan) Trainium2 Bass kernel.
# Self-contained: hardcodes shapes for B=8, L=256, D=384, E=768, N=16, R=24.
# Sharding: pure data-parallel over batch B (1 sample per NeuronCore, 8 cores).
#
# Layout strategy (per core, one batch sample):
#   * All activations live transposed on-chip: [feature, L] with feature on
#     SBUF partitions.
#   * The SSM state volume is processed in 96 "segments" of 8 e-channels;
#     each segment occupies all 128 partitions as (n, e_lo) = 16x8.
#     Segments are grouped into 12 superblocks of 8; the per-superblock
#     decay (af fwd / ar rev variants, zeroed at each segment's scan-entry
#     column) and drive (dt*x*B, bf16) feed ONE packed scan instruction per
#     direction over [128, 2048].
#   * The scan ISA op is DVE-only, so DVE runs almost nothing else: PE
#     replicates dt|u per segment, ACT exps the decay (a_pe per-partition
#     scale) and drains u, DMA copies af->ar, Pool forms the drive b=u*B;
#     col scans start per 2-seg sub-block as slices land, packed row scans
#     follow the boundary-zero memsets.
#   * Readout: PE identity matmuls pre-sum the 4 directions in PSUM (the
#     C-multiply shrinks 4x), ACT drains, Pool multiplies by C, and
#     per-segment selector matmuls accumulate 0.25 * sum_n into the y PSUM;
#     gating and the output projection retire chunk-by-chunk inside the
#     loop so phase 9 work overlaps the scans.
import os
import sys

for _p in ("/opt/trn_rl_repo", "/root/.axon_site/_ro/trn_rl_repo"):
    if os.path.isdir(_p) and _p not in sys.path:
        sys.path.insert(0, _p)

import numpy as np

import concourse.bass as bass
import concourse.mybir as mybir
from concourse import tile, bacc
from concourse.bass_utils import run_bass_kernel_spmd

# Every ACT function this kernel uses (Exp, Ln, Copy, Identity) lives in the
# 'natural_log_exp_and_others' table set.  The default per-instruction set
# assignment ping-pongs between 'exp_and_others' and 'natural_log' (11 table
# loads, ~1.3us each, on the ACT critical path); restricting the choice to
# the one combined set yields a single load.
_orig_get_tables = bacc.get_activation_tables

def _only_combined_tables(arch):
    t = _orig_get_tables(arch)
    return {k: (v if k == "natural_log_exp_and_others" else type(v)())
            for k, v in t.items()}

bacc.get_activation_tables = _only_combined_tables

B, L, D = 8, 256, 384
E, N, R = 768, 16, 24
GRID = 16          # the L = 16x16 patch grid
NCH = E // 128     # 6 e-chunks of 128
NSEG = E // 8      # 96 segments of 8 e-channels (x 16 n = 128 partitions)
NSB = 12           # superblocks of 8 segments
EPS = 1e-5
F32 = mybir.dt.float32
BF16 = mybir.dt.bfloat16
MULT = mybir.AluOpType.mult
ADD = mybir.AluOpType.add
SUB = mybir.AluOpType.subtract
AF = mybir.ActivationFunctionType

# The scan ISA op only exists on DVE (codegen rejects it on Pool), so DVE
# runs all 4 directional scans and nearly nothing else; every elementwise
# sidekick lives on Pool/ACT and the 4-direction sum rides PE identity
# matmuls so the C-multiply shrinks 4x and lands on Pool.
# d: 0 row-fwd, 1 row-rev, 2 col-fwd, 3 col-rev.  Row scans pack all 8
# segments into one 2D instruction; col scans need a 3D (w, h) view per
# segment (the BIR verifier caps scan APs at 3 dims).

_CACHE = {}


def _tts_scan(eng, out, d0, d1, initial, op0, op1):
    # tensor_tensor_scan allowing multi-dim free APs (the library helper
    # asserts 2D; the recurrence chains across free dims, which is exactly
    # what the permuted scan orders need and is verified on HW).
    return eng.add_instruction(mybir.InstTensorScalarPtr(
        name=eng.bass.get_next_instruction_name(),
        is_tensor_tensor_scan=True, is_scalar_tensor_tensor=True,
        op0=op0, op1=op1,
        ins=[eng.lower_ap(d0), eng.lower_ap_or_imm(initial), eng.lower_ap(d1)],
        outs=[eng.lower_ap(out)]))


def _row_ap(t, d):
    # Row-scan views of a [128, 8*256] eight-segment superblock tile: one
    # scan instruction; segment independence is restored by the zeroed
    # decay column at each segment's scan-entry position.
    return t[:, :] if d == 0 else t[:, ::-1]


def _col_ap(t, base, s, d):
    # Col-scan view of segment s at column offset base within a tile (3D:
    # the grid walked column-major, reversed for d == 3).
    o = base + s * 256
    c = t[:, o:o + 256].rearrange("p (h w) -> p w h", h=GRID)
    return c if d == 2 else c[:, ::-1, ::-1]


def _emit(nc, tc, dp, out_d, reps):
    with tc.tile_pool(name="consts", bufs=1) as cp, \
         tc.tile_pool(name="work", bufs=1) as wp, \
         tc.tile_pool(name="seg", bufs=1) as sp:

        def cload(name, shape, dt=F32):
            t = cp.tile(list(shape), dt, name=name, tag=name)
            nc.sync.dma_start(t[:], dp[name][:, :])
            return t

        # Load order = consumption order: x + transpose identity first so
        # phase 1 starts ~1us in, weights behind them.
        x_rows = []
        for lc in range(2):
            t = cp.tile([128, D], F32, name=f"xr{lc}", tag=f"xr{lc}")
            nc.sync.dma_start(t[:], dp["x"][lc * 128:(lc + 1) * 128, :])
            x_rows.append(t)
        ident = cload("ident", (128, 128))
        w_inx = cload("w_inx", (128, 3 * E), BF16)
        w_xT = cload("w_xT", (128, NCH * 88), BF16)
        w_dtT = cload("w_dtT", (R, E), BF16)
        bdt = cload("bdt", (128, NCH))
        c_sel = cload("c_sel", (16, 128), BF16)
        u_sel = cload("u_sel", (128, 16 * 128), BF16)
        a_pe = cload("a_pe", (128, NSEG))
        red_sel = cload("red_sel", (128, 16 * 128), BF16)
        ident_bf = cload("ident_bf", (128, 128), BF16)
        w_inz = cload("w_inz", (128, 3 * E), BF16)
        w_outT = cload("w_outT", (128, NCH * D), BF16)
        dcol = cload("dcol", (128, NCH))
        gam = cload("gam", (128, D))
        bet = cload("bet", (128, D))
        eps_col = cload("eps_col", (128, 1))
        zcol = cload("zcol", (128, 8))

        xT = [wp.tile([128, L], BF16, name=f"xt{i}", tag=f"xt{i}")
              for i in range(3)]
        xin = [wp.tile([128, L], BF16, name=f"xi{i}", tag=f"xi{i}")
               for i in range(NCH)]
        zrow = [wp.tile([128, L], F32, name=f"zr{i}", tag=f"zr{i}")
                for i in range(NCH)]
        zsil = [wp.tile([128, L], F32, name=f"zs{i}", tag=f"zs{i}")
                for i in range(NCH)]
        dtu = [wp.tile([128, 2 * L], BF16, name=f"du{i}", tag=f"du{i}")
               for i in range(NCH)]
        xdbl = wp.tile([R, L], BF16, name="xdbl", tag="xdbl")
        brow = wp.tile([N, L], BF16, name="brow", tag="brow")
        crow = wp.tile([N, L], BF16, name="crow", tag="crow")
        bpe = wp.tile([128, L], F32, name="bpe", tag="bpe")
        cpe = wp.tile([128, L], BF16, name="cpe", tag="cpe")
        yfin = [wp.tile([128, L], BF16, name=f"yf{i}", tag=f"yf{i}")
                for i in range(NCH)]

        for _rep in range(reps):
            with tc.tile_pool(name="psY", bufs=1, space="PSUM") as pyp:
                # Output-projection accumulators: filled chunk-by-chunk as
                # y chunks retire, so phase 9's matmul work overlaps the
                # scan pipeline instead of trailing it.
                po = [pyp.tile([128, D], F32, name=f"po{i}", tag=f"po{i}")
                      for i in range(2)]

                with tc.tile_pool(name="psA", bufs=4, space="PSUM") as pap:
                    # Phase 1: x^T via PE transpose.
                    for dc in range(3):
                        for lc in range(2):
                            ps = pap.tile([128, 256], F32, name="tmp",
                                          tag="tmp")
                            nc.tensor.transpose(
                                ps[:, 0:128],
                                x_rows[lc][:, dc * 128:(dc + 1) * 128],
                                ident[:])
                            nc.vector.tensor_copy(
                                xT[dc][:, lc * 128:(lc + 1) * 128],
                                ps[:, 0:128])
                    # Phase 2 (x half only): x_inner^T = W_in[:E] @ x^T.
                    # The z half runs inside the phase-7 loop (on the rep
                    # PSUM pool) so it stays off the pipeline-head critical
                    # path and fills idle engine slots.
                    for mc in range(NCH):
                        ps = pap.tile([128, 256], F32, name="tmp", tag="tmp")
                        for kc in range(3):
                            nc.tensor.matmul(
                                ps[:, 0:L],
                                w_inx[:, kc * E + mc * 128:
                                      kc * E + (mc + 1) * 128],
                                xT[kc][:, :], start=(kc == 0), stop=(kc == 2))
                        nc.vector.tensor_copy(xin[mc][:, :], ps[:, 0:L])
                    # Phase 3: x_dbl^T = W_x @ x_inner^T, padded so B/C/dt
                    # rows land at partition 0/32/64 (quadrant alignment).
                    ps = pap.tile([128, 256], F32, name="tmp", tag="tmp")
                    for kc in range(NCH):
                        nc.tensor.matmul(
                            ps[0:88, 0:L],
                            w_xT[:, kc * 88:(kc + 1) * 88],
                            xin[kc][:, :], start=(kc == 0),
                            stop=(kc == NCH - 1))
                    nc.scalar.copy(brow[:, :], ps[0:N, 0:L])
                    nc.scalar.copy(crow[:, :], ps[32:32 + N, 0:L])
                    nc.scalar.copy(xdbl[:, :], ps[64:64 + R, 0:L])
                    # Phase 6: B/C rows replicated to the (n, e_lo) layout.
                    ps = pap.tile([128, 256], F32, name="tmp", tag="tmp")
                    nc.tensor.matmul(ps[:, 0:L], c_sel[:], brow[:, :],
                                     start=True, stop=True)
                    nc.vector.tensor_copy(bpe[:, :], ps[:, 0:L])
                    ps = pap.tile([128, 256], F32, name="tmp", tag="tmp")
                    nc.tensor.matmul(ps[:, 0:L], c_sel[:], crow[:, :],
                                     start=True, stop=True)
                    nc.vector.tensor_copy(cpe[:, :], ps[:, 0:L])

                # Phase 7, software-pipelined per superblock q (8 segments):
                # produce(q): PE replicates A*dt and u=dt*x per 2-seg
                # sub-block; ACT exps the decay (fwd variant); DMA engines
                # drain the u replication PSUM->SBUF and copy af->ar (Pool
                # cannot touch PSUM and ACT is saturated); Pool zeroes the
                # scan-entry columns and forms the drive b = u*B; DVE runs
                # the 4 directional scans (the scan ISA op is DVE-only).
                # consume(q-1), per 2-seg quarter: PE sums the 4 directions
                # with identity matmuls into a PSUM accumulator, DMA drains
                # it to SBUF, Pool multiplies by C (bf16 out), PE reduces
                # sum_n into the y PSUM.
                with tc.tile_pool(name="psRep", bufs=2, space="PSUM") as prp, \
                     tc.tile_pool(name="psH", bufs=2, space="PSUM") as php, \
                     tc.tile_pool(name="psYB", bufs=2, space="PSUM") as ybp:
                    bpe2b = bpe[:, :].unsqueeze(1).broadcast_to((128, 2, L))
                    cpe2 = cpe[:, :].unsqueeze(1).broadcast_to((128, 2, L))
                    hb_prev = {}
                    yb_live = {}

                    def produce(q, rows_first=False):
                        # Latency-ordered: each 2-seg sub-block finishes its
                        # decay/drive slices and immediately feeds its two
                        # per-seg col scans (which need no boundary zeros:
                        # a per-segment scan resets state at instr start, so
                        # the zeroed columns only matter for the packed row
                        # scans emitted last).  This keeps DVE rolling while
                        # later sub-blocks are still in flight on PE/ACT/
                        # Pool, closing the produce-chain latency bubble.
                        af = sp.tile([128, 2048], F32, name="af", tag="af",
                                     bufs=2)
                        ar = sp.tile([128, 2048], F32, name="ar", tag="ar",
                                     bufs=2)
                        ubf = sp.tile([128, 2048], BF16, name="ubf",
                                      tag="ubf", bufs=2)
                        bs2 = sp.tile([128, 2048], F32, name="bs2",
                                      tag="bs2", bufs=2)
                        hbig = sp.tile([128, 4 * 2048], BF16, name="hbig",
                                       tag="hbig", bufs=2)
                        c = q // 2
                        for sb in range(4):
                            for s2 in range(2):
                                seg = 8 * q + 2 * sb + s2
                                j = seg % 16
                                sl = 2 * sb + s2
                                rep = prp.tile([128, 512], F32, name="rep",
                                               tag="rep")
                                nc.tensor.matmul(
                                    rep[:, :],
                                    u_sel[:, j * 128:(j + 1) * 128],
                                    dtu[c][:, :], start=True, stop=True)
                                nc.scalar.activation(
                                    af[:, sl * 256:(sl + 1) * 256],
                                    rep[:, 0:256], AF.Exp,
                                    scale=a_pe[:, seg:seg + 1])
                                nc.scalar.copy(
                                    ubf[:, sl * 256:(sl + 1) * 256],
                                    rep[:, 256:512])
                            o = sb * 512
                            bsv = bs2[:, o:o + 512].rearrange(
                                "p (s l) -> p s l", s=2)
                            nc.gpsimd.tensor_tensor(
                                bsv, ubf[:, o:o + 512].rearrange(
                                    "p (s l) -> p s l", s=2),
                                bpe2b, MULT)
                            nc.sync.dma_start(ar[:, o:o + 512],
                                              af[:, o:o + 512])
                            if not rows_first and sb < 3:
                                for s2 in range(2):
                                    s = 2 * sb + s2
                                    _tts_scan(nc.vector,
                                              _col_ap(hbig, 2 * 2048, s, 2),
                                              _col_ap(af, 0, s, 2),
                                              _col_ap(bs2, 0, s, 2),
                                              0.0, MULT, ADD)
                                    _tts_scan(nc.vector,
                                              _col_ap(hbig, 3 * 2048, s, 3),
                                              _col_ap(ar, 0, s, 3),
                                              _col_ap(bs2, 0, s, 3),
                                              0.0, MULT, ADD)
                        # reverse-variant decay: one whole-tile DMA copy,
                        # then zero each variant's scan-entry column (fwd
                        # enters each segment at t=0, rev at t=255; the
                        # zeros multiply h_init=0).  Col-rev scans run
                        # after the copy lands; their per-segment state
                        # reset makes the zero columns irrelevant to them.
                        afs = af[:, :].rearrange("p (s l) -> p s l", s=8)
                        ars = ar[:, :].rearrange("p (s l) -> p s l", s=8)
                        nc.vector.memset(afs[:, :, 0:1], 0.0)
                        nc.vector.memset(ars[:, :, L - 1:L], 0.0)
                        for d in (0, 1):
                            asrc = af if d == 0 else ar
                            hb = hbig[:, d * 2048:(d + 1) * 2048]
                            _tts_scan(nc.vector, _row_ap(hb, d),
                                      _row_ap(asrc, d), _row_ap(bs2, d),
                                      0.0, MULT, ADD)
                        if not rows_first:
                            for s in (6, 7):
                                for d in (2, 3):
                                    _tts_scan(nc.vector,
                                              _col_ap(hbig, d * 2048, s, d),
                                              _col_ap(af if d == 2 else ar,
                                                      0, s, d),
                                              _col_ap(bs2, 0, s, d),
                                              0.0, MULT, ADD)
                        if rows_first:
                            # tail superblock: rows first, then cols in
                            # quarter order so the consume chain starts on
                            # quarter 0 while later cols still run.
                            for s in range(8):
                                for d in (2, 3):
                                    _tts_scan(nc.vector,
                                              _col_ap(hbig, d * 2048, s, d),
                                              _col_ap(af if d == 2 else ar,
                                                      0, s, d),
                                              _col_ap(bs2, 0, s, d),
                                              0.0, MULT, ADD)
                        hb_prev[q] = hbig

                    def consume(q):
                        hbig = hb_prev.pop(q)
                        c = q // 2
                        if q % 2 == 0:
                            yb_live[c] = ybp.tile([128, 256], F32,
                                                  name="ybig", tag="ybig")
                        yb = yb_live[c]
                        for k in range(4):
                            hsum = php.tile([128, 512], F32, name="hsum",
                                            tag="hsum")
                            for d in range(4):
                                nc.tensor.matmul(
                                    hsum[:, :], ident_bf[:],
                                    hbig[:, d * 2048 + k * 512:
                                          d * 2048 + (k + 1) * 512],
                                    start=(d == 0), stop=(d == 3))
                            hsb = sp.tile([128, 512], F32, name="hsb",
                                          tag="hsb", bufs=2)
                            nc.scalar.copy(hsb[:, :], hsum[:, :])
                            chs = sp.tile([128, 512], BF16, name="chs",
                                          tag="chs", bufs=2)
                            nc.gpsimd.tensor_tensor(
                                chs[:, :].rearrange("p (s l) -> p s l", s=2),
                                hsb[:, :].rearrange("p (s l) -> p s l", s=2),
                                cpe2, MULT)
                            for s2 in range(2):
                                seg = 8 * q + 2 * k + s2
                                j = seg % 16
                                nc.tensor.matmul(
                                    yb[:, :],
                                    red_sel[:, j * 128:(j + 1) * 128],
                                    chs[:, s2 * 256:(s2 + 1) * 256],
                                    start=(q % 2 == 0 and k == 0
                                           and s2 == 0),
                                    stop=(q % 2 == 1 and k == 3 and s2 == 1),
                                    skip_group_check=True)

                    def phase4(mc):
                        # dt^T = softplus(W_dt @ dtraw^T + b_dt)
                        #      = Ln(Exp(raw + b_dt) + 1); u^T = dt^T * x^T.
                        # Runs inside the superblock loop on the rep PSUM
                        # pool, two superblocks ahead of its consumer.
                        ps = prp.tile([128, 512], F32, name="rep",
                                      tag="rep")
                        nc.tensor.matmul(ps[:, 0:L],
                                         w_dtT[:, mc * 128:(mc + 1) * 128],
                                         xdbl[:, :], start=True, stop=True)
                        spl = sp.tile([128, L], F32, name="spl", tag="spl",
                                      bufs=2)
                        nc.scalar.activation(spl[:, :], ps[:, 0:L], AF.Exp,
                                             bias=bdt[:, mc:mc + 1])
                        nc.scalar.activation(dtu[mc][:, 0:L], spl[:, :],
                                             AF.Ln, bias=1.0)
                        nc.gpsimd.tensor_tensor(dtu[mc][:, L:2 * L],
                                                dtu[mc][:, 0:L],
                                                xin[mc][:, :], MULT)

                    def silu_z(zc):
                        # z half of the input projection + silu, run inside
                        # the loop: zsil = z / (1 + exp(-z)).
                        ps = prp.tile([128, 512], F32, name="rep",
                                      tag="rep")
                        for kc in range(3):
                            nc.tensor.matmul(
                                ps[:, 0:L],
                                w_inz[:, kc * E + zc * 128:
                                      kc * E + (zc + 1) * 128],
                                xT[kc][:, :], start=(kc == 0), stop=(kc == 2))
                        nc.scalar.copy(zrow[zc][:, :], ps[:, 0:L])
                        ez = sp.tile([128, L], F32, name="ez", tag="ez",
                                     bufs=2)
                        nc.scalar.activation(ez[:, :], ps[:, 0:L],
                                             AF.Exp, scale=-1.0)
                        nc.gpsimd.tensor_scalar_add(ez[:, :], ez[:, :], 1.0)
                        sg = sp.tile([128, L], F32, name="sg", tag="sg",
                                     bufs=2)
                        nc.vector.reciprocal(sg[:, :], ez[:, :])
                        nc.gpsimd.tensor_tensor(zsil[zc][:, :],
                                                zrow[zc][:, :],
                                                sg[:, :], MULT)

                    def phase8(c):
                        # y_fin^T = y^T * silu(z^T) + x_inner^T * D, then
                        # fold this chunk into the output projection.
                        yb = yb_live.pop(c)
                        t1 = sp.tile([128, L], F32, name="fin", tag="fin",
                                     bufs=2)
                        nc.vector.tensor_tensor(t1[:, :], yb[:, :],
                                                zsil[c][:, :], MULT)
                        nc.vector.scalar_tensor_tensor(
                            yfin[c][:, :], xin[c][:, :], dcol[:, c:c + 1],
                            t1[:, :], MULT, ADD)
                        for lc in range(2):
                            nc.tensor.matmul(po[lc][:, :],
                                             yfin[c][:, lc * 128:
                                                     (lc + 1) * 128],
                                             w_outT[:, c * D:(c + 1) * D],
                                             start=(c == 0),
                                             stop=(c == NCH - 1))

                    phase4(0)
                    phase4(1)
                    for q in range(NSB + 1):
                        if q < NSB:
                            produce(q, rows_first=(q == NSB - 1))
                        if q >= 1:
                            consume(q - 1)
                            if (q - 1) % 2 == 1:
                                phase8((q - 1) // 2)
                        if q < NSB:
                            if q % 2 == 0 and q // 2 + 2 < NCH:
                                phase4(q // 2 + 2)
                            if q % 2 == 0:
                                silu_z(q // 2)

                # Phase 9/10: residual, layernorm, store (the projection
                # itself accumulated into po[] inside the loop).
                for lc in range(2):
                    o1 = sp.tile([128, D], F32, name="o1", tag="o1", bufs=2)
                    s1 = sp.tile([128, 1], F32, name="st", tag="st", bufs=8)
                    nc.vector.scalar_tensor_tensor(o1[:, :], po[lc][:, :],
                                                   0.0,
                                                   x_rows[lc][:, :], ADD, ADD,
                                                   accum_out=s1[:, :])
                    sq = sp.tile([128, D], F32, name="sq", tag="sq", bufs=2)
                    s2 = sp.tile([128, 1], F32, name="st", tag="st", bufs=8)
                    nc.vector.scalar_tensor_tensor(sq[:, :], o1[:, :], 0.0,
                                                   o1[:, :], ADD, MULT,
                                                   accum_out=s2[:, :])
                    mu = sp.tile([128, 1], F32, name="st", tag="st", bufs=8)
                    nc.vector.tensor_scalar_mul(mu[:, :], s1[:, :], 1.0 / D)
                    ex2 = sp.tile([128, 1], F32, name="st", tag="st", bufs=8)
                    nc.vector.tensor_scalar_mul(ex2[:, :], s2[:, :], 1.0 / D)
                    var = sp.tile([128, 1], F32, name="st", tag="st", bufs=8)
                    nc.vector.scalar_tensor_tensor(var[:, :], mu[:, :], 0.0,
                                                   mu[:, :], ADD, MULT)
                    nc.vector.tensor_sub(var[:, :], ex2[:, :], var[:, :])
                    lv = sp.tile([128, 1], F32, name="st", tag="st", bufs=8)
                    nc.scalar.activation(lv[:, :], var[:, :], AF.Ln,
                                         bias=eps_col[:, :])
                    rstd = sp.tile([128, 1], F32, name="st", tag="st", bufs=8)
                    nc.scalar.activation(rstd[:, :], lv[:, :], AF.Exp,
                                         scale=-0.5)
                    t2 = sp.tile([128, D], F32, name="t2", tag="t2", bufs=2)
                    nc.vector.scalar_tensor_tensor(t2[:, :], o1[:, :],
                                                   mu[:, :], gam[:, :],
                                                   SUB, MULT)
                    orow = sp.tile([128, D], F32, name="orow", tag="orow",
                                   bufs=2)
                    nc.vector.scalar_tensor_tensor(orow[:, :], t2[:, :],
                                                   rstd[:, :], bet[:, :],
                                                   MULT, ADD)
                    nc.sync.dma_start(out_d[lc * 128:(lc + 1) * 128, :],
                                      orow[:, :])


def _build(reps=1):
    key = ("nc", reps)
    if key in _CACHE:
        return _CACHE[key]
    nc = bacc.Bacc("TRN2", target_bir_lowering=False, debug=False,
                   num_devices=8)

    dp = {}
    def din(name, shape, dt=F32):
        dp[name] = nc.dram_tensor(name, list(shape), dt, kind="ExternalInput")

    din("x", (L, D))
    din("w_inx", (128, 3 * E), BF16)
    din("w_inz", (128, 3 * E), BF16)
    din("w_xT", (128, NCH * 88), BF16)
    din("w_dtT", (R, E), BF16)
    din("w_outT", (128, NCH * D), BF16)
    din("u_sel", (128, 16 * 128), BF16)
    din("a_pe", (128, NSEG))
    din("c_sel", (16, 128), BF16)
    din("red_sel", (128, 16 * 128), BF16)
    din("bdt", (128, NCH))
    din("dcol", (128, NCH))
    din("gam", (128, D))
    din("bet", (128, D))
    din("ident", (128, 128))
    din("ident_bf", (128, 128), BF16)
    din("eps_col", (128, 1))
    din("zcol", (128, 8))
    out_d = nc.dram_tensor("out", [L, D], F32, kind="ExternalOutput")

    with tile.TileContext(nc) as tc:
        _emit(nc, tc, dp, out_d, reps)

    nc.compile()
    _CACHE[key] = nc
    return nc


def _host_prep(W_in, A_log, W_x, W_dt, b_dt, D_param, W_out, gamma, beta):
    import ml_dtypes
    f = np.float32
    w_in3 = W_in.T.reshape(3, 128, 2 * E).transpose(1, 0, 2)  # (128,3,2E)
    w_inx = np.ascontiguousarray(
        w_in3[:, :, 0:E].reshape(128, 3 * E)).astype(ml_dtypes.bfloat16)
    w_inz = np.ascontiguousarray(
        w_in3[:, :, E:2 * E].reshape(128, 3 * E)).astype(ml_dtypes.bfloat16)
    wxt = np.asarray(W_x.T, f)                       # (E, 56)
    wxt_pad = np.zeros((E, 88), f)
    wxt_pad[:, 0:N] = wxt[:, R:R + N]                # B rows -> 0
    wxt_pad[:, 32:32 + N] = wxt[:, R + N:R + 2 * N]  # C rows -> 32
    wxt_pad[:, 64:64 + R] = wxt[:, 0:R]              # dt rows -> 64
    w_xT = np.ascontiguousarray(
        wxt_pad.reshape(NCH, 128, 88).transpose(1, 0, 2).reshape(
            128, NCH * 88)).astype(ml_dtypes.bfloat16)
    w_dtT = np.ascontiguousarray(W_dt.T).astype(ml_dtypes.bfloat16)
    w_outT = np.ascontiguousarray(
        W_out.T.reshape(NCH, 128, D).transpose(1, 0, 2).reshape(
            128, NCH * D)).astype(ml_dtypes.bfloat16)
    u_sel = np.zeros((128, 16 * 128), ml_dtypes.bfloat16)
    c_sel = np.zeros((16, 128), ml_dtypes.bfloat16)
    for n in range(16):
        for elo in range(8):
            c_sel[n, n * 8 + elo] = 1.0
            for j in range(16):
                u_sel[8 * j + elo, j * 128 + n * 8 + elo] = 1.0
    A = -np.exp(np.asarray(A_log, np.float64))          # (E, N)
    a_pe = np.zeros((128, NSEG), f)
    for ehi in range(NSEG):
        for n in range(16):
            for elo in range(8):
                a_pe[n * 8 + elo, ehi] = A[8 * ehi + elo, n]
    red_sel = np.zeros((128, 16 * 128), ml_dtypes.bfloat16)
    for j in range(16):
        for n in range(16):
            for elo in range(8):
                red_sel[n * 8 + elo, j * 128 + 8 * j + elo] = 0.25
    bdt = np.ascontiguousarray(np.asarray(b_dt, f).reshape(NCH, 128).T)
    dcol = np.ascontiguousarray(np.asarray(D_param, f).reshape(NCH, 128).T)
    gam = np.ascontiguousarray(np.broadcast_to(np.asarray(gamma, f), (128, D)))
    bet = np.ascontiguousarray(np.broadcast_to(np.asarray(beta, f), (128, D)))
    ident = np.eye(128, dtype=f)
    ident_bf = np.eye(128, dtype=ml_dtypes.bfloat16)
    eps_col = np.full((128, 1), EPS, f)
    zcol = np.zeros((128, 8), f)
    return dict(w_inx=w_inx, w_inz=w_inz, w_xT=w_xT, w_dtT=w_dtT, w_outT=w_outT,
                u_sel=u_sel, a_pe=a_pe, c_sel=c_sel, red_sel=red_sel,
                bdt=bdt, dcol=dcol, gam=gam, bet=bet, ident=ident,
                ident_bf=ident_bf, eps_col=eps_col, zcol=zcol)


def kernel(x, W_in, A_log, W_x, W_dt, b_dt, D_param, W_out, gamma, beta):
    x = np.asarray(x, np.float32)
    common = _host_prep(W_in, A_log, W_x, W_dt, b_dt, D_param, W_out,
                        gamma, beta)
    in_maps = [dict(common, x=np.ascontiguousarray(x[b])) for b in range(B)]
    nc = _build()
    res = run_bass_kernel_spmd(nc, in_maps, list(range(B)))
    return np.stack([res.results[b]["out"] for b in range(B)], axis=0)


def _pjrt_runner(nc, in_maps):
    """Device-resident repeat runner for one prebuilt program."""
    import jax
    import jax.numpy as jnp
    from jax.sharding import Mesh, PartitionSpec
    try:
        from jax.experimental.shard_map import shard_map
    except ImportError:
        from jax.shard_map import shard_map
    from concourse import bass2jax
    bass2jax.install_neuronx_cc_hook()

    in_names, out_names, out_avals = [], [], []
    pname = nc.partition_id_tensor.name if nc.partition_id_tensor else None
    for alloc in nc.m.functions[0].allocations:
        if not isinstance(alloc, mybir.MemoryLocationSet):
            continue
        name = alloc.memorylocations[0].name
        if alloc.kind == "ExternalInput":
            if name != pname:
                in_names.append(name)
        elif alloc.kind == "ExternalOutput":
            out_names.append(name)
            out_avals.append(jax.core.ShapedArray(
                tuple(alloc.tensor_shape), mybir.dt.np(alloc.dtype)))
    bind_names = list(in_names) + list(out_names)
    if pname is not None:
        bind_names = bind_names + [pname]

    def f(*args):
        operands = list(args)
        if pname is not None:
            operands.append(bass2jax.partition_id_tensor())
        return tuple(bass2jax._bass_exec_p.bind(
            *operands, out_avals=tuple(out_avals), in_names=tuple(bind_names),
            out_names=tuple(out_names), lowering_input_output_aliases=(),
            sim_require_finite=True, sim_require_nnan=True, nc=nc))

    devices = jax.devices()[:B]
    mesh = Mesh(np.asarray(devices), ("core",))
    nargs = len(in_names) + len(out_avals)
    jf = jax.jit(shard_map(f, mesh=mesh,
                           in_specs=(PartitionSpec("core"),) * nargs,
                           out_specs=(PartitionSpec("core"),) * len(out_names),
                           check_rep=False))
    concat_in = [np.concatenate([np.asarray(m[nm]) for m in in_maps], axis=0)
                 for nm in in_names]
    for av in out_avals:
        concat_in.append(np.zeros((B * av.shape[0],) + tuple(av.shape[1:]),
                                  av.dtype))
    dev_in = [jax.device_put(a) for a in concat_in]
    return jf, dev_in


def timed_chain(inputs, n_iters=4, n_warm=1):
    """Per-iteration HW time: the same program body emitted `reps` times;
    wall(reps=n_iters) - wall(reps=n_warm) cancels dispatch overhead."""
    import time
    import jax
    x = np.asarray(inputs["x"], np.float32)
    common = _host_prep(**{k: np.asarray(inputs[k]) for k in
        ("W_in", "A_log", "W_x", "W_dt", "b_dt", "D_param", "W_out",
         "gamma", "beta")})
    in_maps = [dict(common, x=np.ascontiguousarray(x[b])) for b in range(B)]

    def measure(reps):
        nc = _build(reps)
        jf, dev_in = _pjrt_runner(nc, in_maps)
        r = jf(*dev_in)
        jax.block_until_ready(r)
        best = float("inf")
        for _ in range(6):
            t0 = time.perf_counter()
            r = jf(*dev_in)
            jax.block_until_ready(r)
            best = min(best, time.perf_counter() - t0)
        return best

    t_long = measure(n_iters)
    t_short = measure(n_warm)
    return (t_long - t_short) / (n_iters - n_warm) * 1e9
